# revision 17
# baseline (speedup 1.0000x reference)
"""Depthwise causal Conv1d (k=4) + SiLU on 8 Trainium2 NeuronCores.

Problem: x [4, 4096, 2048] f32, w [2048, 4] f32,
out[b, t, d] = silu(sum_j w[d, j] * x[b, t - 3 + j, d])   (zero-padded left).

Sharding: 8 cores = 4 batches x 2 channel-halves. Depthwise conv is
independent per channel, so channel sharding needs no halo exchange.

Layout: each core receives its shard host-transposed to [channels, time]
(channels on SBUF partitions). The per-channel weight w[d, j] is then a
per-partition scalar and the causal time shifts are free-dim AP offsets
into one loaded tile.

Precision: x and the output are host-cast fp16 (halves HBM traffic both
ways); products and the add tree stay fp16 (PE accumulates fp32 in
PSUM); SiLU computes fp32-internally on ACT. Rel err ~5e-4.

Schedule (DMA-bound problem: ~16.8 MB/core over 16 DMA engines):
 - All 8 channel-block rows of x are loaded up-front into SBUF (fits:
   ~66 KB/partition) so the 16 DMA engines always have load work queued
   and compute never starves. Loads issue on SyncE (HWDGE).
 - 5 blocks run on the TensorEngine as diag(w_j) matmuls accumulating
   the 4 taps in PSUM (LDWEIGHTS is pipelined with the previous matmul,
   so PE costs ~8 us/block); 3 blocks run on DVE as 4 tensor_scalar
   products (1-src ops hit the 2x fp16 mode, ~3 elem/ns) + 3 plain
   non-aliased adds (2-src ops run ~1.8 elem/ns; fused
   scalar_tensor_tensor measured slower at ~0.9 elem/ns).
 - ACT does SiLU only, per 2048-col chunk (PE chunks straight out of
   PSUM), so output chunks flow continuously instead of bursting.
 - All stores issue on GpSimd (SWDGE): its rings are separate from the
   HWDGE load rings, so the DMA engines round-robin loads and stores
   instead of draining the whole load backlog first.
"""

import sys
import types

import numpy as np

import concourse.bass as bass
import concourse.bacc as bacc
import concourse.mybir as mybir
from concourse.tile import TileContext
from concourse.bass_utils import run_bass_kernel_spmd


def _ensure_ntff_hook():
    """bass_utils imports antenv.axon_hooks when BASS_TRACE is set; that
    module is absent on this image. Install a shim so tracing works when
    possible and degrades gracefully (instead of crashing) when not."""
    try:
        import antenv.axon_hooks  # noqa: F401

        return
    except ImportError:
        pass
    try:
        import antenv

        hook = None
        try:
            if "/root/.axon_site" not in sys.path:
                sys.path.insert(0, "/root/.axon_site")
            from trn_agent_boot.trn_boot import _ntff_profile_via_ctypes

            hook = _ntff_profile_via_ctypes("/opt/axon/libaxon_pjrt.so")
        except Exception:
            hook = None
        mod = types.ModuleType("antenv.axon_hooks")
        mod._hook = hook
        mod.get_axon_ntff_profile_hook = lambda: mod._hook
        mod.set_axon_ntff_profile_hook = lambda h: setattr(mod, "_hook", h)
        sys.modules["antenv.axon_hooks"] = mod
        antenv.axon_hooks = mod
    except Exception:
        pass


_ensure_ntff_hook()

B, L, D = 4, 4096, 2048
K = 4
PAD = K - 1
N_CORES = 8
DH = D // 2            # channels per core
NBLK = DH // 128       # 128-partition channel blocks per core
ROWW = 4128            # DRAM row stride (fp16 elems): 64B-aligned rows

MID_DT = mybir.dt.float16
D_BLKS = [0, 2, 4]     # DVE vector-path blocks
P_BLKS = [1, 3, 5, 6, 7]  # TensorEngine (diag matmul) blocks
CH = 2048              # compute/store chunk (cols)

_cache = {}


def _build_bass():
    nc = bacc.Bacc()
    xt = nc.dram_tensor("xt", [DH, ROWW], MID_DT, kind="ExternalInput")
    wt = nc.dram_tensor("wt", [128, NBLK * K], mybir.dt.float32, kind="ExternalInput")
    # diag(w) blocks for the PE path, packed for P_BLKS only: partition p,
    # col (pi*K + j)*128 + m holds w[P_BLKS[pi]*128 + p, j] iff m == p else 0
    wd = nc.dram_tensor(
        "wd", [128, len(P_BLKS) * K * 128], MID_DT, kind="ExternalInput"
    )
    ot = nc.dram_tensor("ot", [DH, L], MID_DT, kind="ExternalOutput")
    f32 = mybir.dt.float32

    with TileContext(nc) as tc:
        with tc.tile_pool(name="pool", bufs=2) as pool, \
             tc.tile_pool(name="psum", bufs=2, space="PSUM") as psum_pool:
            # Warmup: a tiny Silu forces the silu activation-table set to
            # load during the initial DMA wait; it is the only table load
            # in the whole kernel.
            warm = pool.tile([128, 2], MID_DT, tag="warm", bufs=1)
            nc.vector.memset(warm[:], 0.0)
            nc.scalar.activation(warm[:], warm[:], mybir.ActivationFunctionType.Silu)

            w = pool.tile([128, NBLK * K], f32, tag="w", bufs=1)
            nc.sync.dma_start(out=w[:], in_=wt[:, :])

            wdt = pool.tile([128, len(P_BLKS) * K * 128], MID_DT, tag="wd", bufs=1)
            xtile = {}
            # blocks 0 and 1 load a small first quarter, and the first PE
            # block's diag slice loads separately, so both compute engines
            # start as early as possible.
            Q = 1024
            x0 = pool.tile([128, L + PAD + 1], MID_DT, tag="x0", bufs=1)
            x1 = pool.tile([128, L + PAD + 1], MID_DT, tag="x1", bufs=1)
            xtile[0], xtile[1] = x0, x1
            nc.sync.dma_start(out=x0[:, 0 : Q + PAD], in_=xt[0:128, 0 : Q + PAD])
            nc.sync.dma_start(out=x1[:, 0 : Q + PAD], in_=xt[128:256, 0 : Q + PAD])
            nc.sync.dma_start(out=wdt[:, 0 : K * 128], in_=wd[:, 0 : K * 128])
            nc.sync.dma_start(
                out=x0[:, Q + PAD : L + PAD], in_=xt[0:128, Q + PAD : L + PAD]
            )
            nc.sync.dma_start(
                out=x1[:, Q + PAD : L + PAD], in_=xt[128:256, Q + PAD : L + PAD]
            )
            nc.sync.dma_start(out=wdt[:, K * 128 :], in_=wd[:, K * 128 :])
            for blk in range(2, NBLK):
                xb = pool.tile([128, L + PAD + 1], MID_DT, tag=f"x{blk}", bufs=1)
                nc.sync.dma_start(
                    out=xb[:, 0 : L + PAD],
                    in_=xt[blk * 128 : (blk + 1) * 128, 0 : L + PAD],
                )
                xtile[blk] = xb

            def d_unit(blk, t0, tl):
                # products shift-rebased (m_j[:, t] = w_j * x[:, t + j]) so
                # the fp16 add tree stays aligned; plain non-aliased adds.
                x = xtile[blk]
                r0 = blk * 128
                wj = lambda j: w[:, blk * K + j : blk * K + j + 1]
                m = pool.tile([128, 4, CH], MID_DT, tag="m", bufs=2)
                for j in range(K):
                    nc.vector.tensor_scalar_mul(
                        m[:, j, 0:tl], x[:, t0 + j : t0 + j + tl], wj(j)
                    )
                a = pool.tile([128, 3, CH], MID_DT, tag="a", bufs=3)
                nc.vector.tensor_add(a[:, 0, 0:tl], m[:, 0, 0:tl], m[:, 1, 0:tl])
                nc.vector.tensor_add(a[:, 1, 0:tl], m[:, 2, 0:tl], m[:, 3, 0:tl])
                nc.vector.tensor_add(a[:, 2, 0:tl], a[:, 0, 0:tl], a[:, 1, 0:tl])
                o = pool.tile([128, CH], MID_DT, tag="o", bufs=6)
                # silu in 1024-col pieces: uniform ACT granularity caps
                # head-of-line blocking of the PE path's PSUM drain
                for h in range(tl // 1024):
                    nc.scalar.activation(
                        o[:, h * 1024 : (h + 1) * 1024],
                        a[:, 2, h * 1024 : (h + 1) * 1024],
                        mybir.ActivationFunctionType.Silu,
                    )
                nc.gpsimd.dma_start(out=ot[r0 : r0 + 128, t0 : t0 + tl], in_=o[:, 0:tl])

            def p_unit(blk, t0, tl):
                x = xtile[blk]
                r0 = blk * 128
                pi = P_BLKS.index(blk)
                # 1024-col PSUM tiles (2 banks) x 4 bufs: PE runs up to 4
                # tiles ahead of ACT's PSUM drain.
                o = pool.tile([128, CH], MID_DT, tag="o", bufs=6)
                for h in range(tl // 1024):
                    ps = psum_pool.tile([128, 1024], f32, tag="ps", bufs=4)
                    for c in range(2):
                        for j in range(K):
                            lw = wdt[:, (pi * K + j) * 128 : (pi * K + j + 1) * 128]
                            b0 = t0 + h * 1024 + c * 512
                            nc.tensor.matmul(
                                ps[:, c * 512 : (c + 1) * 512],
                                lw,
                                x[:, b0 + j : b0 + j + 512],
                                start=(j == 0),
                                stop=(j == K - 1),
                            )
                    nc.scalar.activation(
                        o[:, h * 1024 : (h + 1) * 1024],
                        ps[:],
                        mybir.ActivationFunctionType.Silu,
                    )
                nc.gpsimd.dma_start(out=ot[r0 : r0 + 128, t0 : t0 + tl], in_=o[:, 0:tl])

            # chunk stream ordered by estimated completion time (DVE ~3 us
            # per 1024 cols, PE ~1.85 us) so ACT's in-order SiLU queue
            # matches production order and neither engine head-of-line
            # blocks the other's drain. 1024-col units at the stream edges
            # start compute sooner and drain the tail faster; DVE assists
            # with block 7's last quarter so both engines finish together.
            stream = [
                (d_unit, 0, 0, Q), (p_unit, 1, 0, Q), (d_unit, 0, Q, Q),
                (p_unit, 1, Q, Q), (p_unit, 1, CH, CH), (d_unit, 0, CH, CH),
                (p_unit, 3, 0, CH), (d_unit, 2, 0, CH), (p_unit, 3, CH, CH),
                (p_unit, 5, 0, CH), (d_unit, 2, CH, CH), (p_unit, 5, CH, CH),
                (d_unit, 4, 0, CH), (p_unit, 6, 0, CH), (p_unit, 6, CH, CH),
                (d_unit, 4, CH, CH), (p_unit, 7, 0, CH), (p_unit, 7, CH, Q),
                (d_unit, 7, CH + Q, Q),
            ]
            for fn, blk, t0, tl in stream:
                fn(blk, t0, tl)
    nc.compile()
    return nc


def _shard_inputs(x, w):
    in_maps = []
    for core in range(N_CORES):
        b, half = divmod(core, 2)
        d0 = half * DH
        xt = np.zeros((DH, ROWW), dtype=np.float16)
        xt[:, PAD : PAD + L] = x[b, :, d0 : d0 + DH].T.astype(np.float16)
        # w rows for this shard, rearranged so partition p holds the K
        # weights of channel blk*128 + p at free cols [blk*K, blk*K + K)
        w_sh = w[d0 : d0 + DH].reshape(NBLK, 128, K)
        wt = (
            w_sh.transpose(1, 0, 2).reshape(128, NBLK * K).astype(np.float32)
        )
        # diag blocks for the PE path (P_BLKS only)
        wdv = np.zeros((128, len(P_BLKS), K, 128), dtype=np.float16)
        idx = np.arange(128)
        wdv[idx, :, :, idx] = w_sh[P_BLKS].transpose(1, 0, 2).astype(np.float16)
        in_maps.append(
            {
                "xt": np.ascontiguousarray(xt),
                "wt": np.ascontiguousarray(wt),
                "wd": np.ascontiguousarray(
                    wdv.reshape(128, len(P_BLKS) * K * 128)
                ),
            }
        )
    return in_maps


def kernel(x, w):
    x = np.asarray(x, dtype=np.float32)
    w = np.asarray(w, dtype=np.float32)
    assert x.shape == (B, L, D) and w.shape == (D, K)

    if "nc" not in _cache:
        _cache["nc"] = _build_bass()
    nc = _cache["nc"]

    in_maps = _shard_inputs(x, w)
    res = None
    for attempt in range(3):
        try:
            res = run_bass_kernel_spmd(nc, in_maps, core_ids=list(range(N_CORES)))
            break
        except Exception:
            if attempt == 2:
                raise
    _cache["last_results"] = res

    out = np.empty((B, L, D), dtype=np.float32)
    for core in range(N_CORES):
        b, half = divmod(core, 2)
        d0 = half * DH
        out[b, :, d0 : d0 + DH] = res.results[core]["ot"].T.astype(np.float32)
    return out


# revision 24
# speedup vs baseline: 1.0228x; 1.0228x over previous
"""Depthwise causal Conv1d (k=4) + SiLU on 8 Trainium2 NeuronCores.

Problem: x [4, 4096, 2048] f32, w [2048, 4] f32,
out[b, t, d] = silu(sum_j w[d, j] * x[b, t - 3 + j, d])   (zero-padded left).

Sharding: 8 cores = 4 batches x 2 channel-halves. Depthwise conv is
independent per channel, so channel sharding needs no halo exchange.

Layout: each core receives its shard host-transposed to [channels, time]
(channels on SBUF partitions). The per-channel weight w[d, j] is then a
per-partition scalar and the causal time shifts are free-dim AP offsets
into one loaded tile.

Precision: x and the output are host-cast fp16 (halves HBM traffic both
ways); products and the add tree stay fp16 (PE accumulates fp32 in
PSUM); SiLU computes fp32-internally on ACT. Rel err ~5e-4.

Schedule (DMA-bound problem: ~16.8 MB/core over 16 DMA engines):
 - All 8 channel-block rows of x are loaded up-front into SBUF (fits:
   ~66 KB/partition) so the 16 DMA engines always have load work queued
   and compute never starves. Loads issue on SyncE (HWDGE).
 - 5 blocks run on the TensorEngine as diag(w_j) matmuls accumulating
   the 4 taps in PSUM (LDWEIGHTS is pipelined with the previous matmul,
   so PE costs ~8 us/block); 3 blocks run on DVE as 4 tensor_scalar
   products (1-src ops hit the 2x fp16 mode, ~3 elem/ns) + 3 plain
   non-aliased adds (2-src ops run ~1.8 elem/ns; fused
   scalar_tensor_tensor measured slower at ~0.9 elem/ns).
 - ACT does SiLU only, per 2048-col chunk (PE chunks straight out of
   PSUM), so output chunks flow continuously instead of bursting.
 - All stores issue on GpSimd (SWDGE): its rings are separate from the
   HWDGE load rings, so the DMA engines round-robin loads and stores
   instead of draining the whole load backlog first.
"""

import sys
import types

import numpy as np

import concourse.bass as bass
import concourse.bacc as bacc
import concourse.mybir as mybir
from concourse.tile import TileContext
from concourse.bass_utils import run_bass_kernel_spmd


def _ensure_ntff_hook():
    """bass_utils imports antenv.axon_hooks when BASS_TRACE is set; that
    module is absent on this image. Install a shim so tracing works when
    possible and degrades gracefully (instead of crashing) when not."""
    try:
        import antenv.axon_hooks  # noqa: F401

        return
    except ImportError:
        pass
    try:
        import antenv

        hook = None
        try:
            if "/root/.axon_site" not in sys.path:
                sys.path.insert(0, "/root/.axon_site")
            from trn_agent_boot.trn_boot import _ntff_profile_via_ctypes

            hook = _ntff_profile_via_ctypes("/opt/axon/libaxon_pjrt.so")
        except Exception:
            hook = None
        mod = types.ModuleType("antenv.axon_hooks")
        mod._hook = hook
        mod.get_axon_ntff_profile_hook = lambda: mod._hook
        mod.set_axon_ntff_profile_hook = lambda h: setattr(mod, "_hook", h)
        sys.modules["antenv.axon_hooks"] = mod
        antenv.axon_hooks = mod
    except Exception:
        pass


_ensure_ntff_hook()

B, L, D = 4, 4096, 2048
K = 4
PAD = K - 1
N_CORES = 8
DH = D // 2            # channels per core
NBLK = DH // 128       # 128-partition channel blocks per core
ROWW = 4128            # DRAM row stride (fp16 elems): 64B-aligned rows

MID_DT = mybir.dt.float16
D_BLKS = [0, 2, 4]     # DVE vector-path blocks
P_BLKS = [1, 3, 5, 6, 7]  # TensorEngine (diag matmul) blocks
CH = 2048              # compute/store chunk (cols)

_cache = {}


def _build_bass():
    nc = bacc.Bacc()
    xt = nc.dram_tensor("xt", [DH, ROWW], MID_DT, kind="ExternalInput")
    wt = nc.dram_tensor("wt", [128, NBLK * K], mybir.dt.float32, kind="ExternalInput")
    # diag(w) blocks for the PE path, packed for P_BLKS only: partition p,
    # col (pi*K + j)*128 + m holds w[P_BLKS[pi]*128 + p, j] iff m == p else 0
    wd = nc.dram_tensor(
        "wd", [128, len(P_BLKS) * K * 128], MID_DT, kind="ExternalInput"
    )
    ot = nc.dram_tensor("ot", [DH, L], MID_DT, kind="ExternalOutput")
    f32 = mybir.dt.float32

    with TileContext(nc) as tc:
        with tc.tile_pool(name="pool", bufs=2) as pool, \
             tc.tile_pool(name="psum", bufs=2, space="PSUM") as psum_pool:
            # Warmup: a tiny Silu forces the silu activation-table set to
            # load during the initial DMA wait; it is the only table load
            # in the whole kernel.
            warm = pool.tile([128, 2], MID_DT, tag="warm", bufs=1)
            nc.vector.memset(warm[:], 0.0)
            nc.scalar.activation(warm[:], warm[:], mybir.ActivationFunctionType.Silu)

            w = pool.tile([128, NBLK * K], f32, tag="w", bufs=1)
            nc.sync.dma_start(out=w[:], in_=wt[:, :])

            wdt = pool.tile([128, len(P_BLKS) * K * 128], MID_DT, tag="wd", bufs=1)
            xtile = {}
            # blocks 0 and 1 load a small first quarter, and the first PE
            # block's diag slice loads separately, so both compute engines
            # start as early as possible.
            Q = 1024
            x0 = pool.tile([128, L + PAD + 1], MID_DT, tag="x0", bufs=1)
            x1 = pool.tile([128, L + PAD + 1], MID_DT, tag="x1", bufs=1)
            xtile[0], xtile[1] = x0, x1
            for blk in range(2, NBLK):
                xb = pool.tile([128, L + PAD + 1], MID_DT, tag=f"x{blk}", bufs=1)
                xtile[blk] = xb

            def load_x(blk, c0, c1):
                nc.sync.dma_start(
                    out=xtile[blk][:, c0:c1],
                    in_=xt[blk * 128 : (blk + 1) * 128, c0:c1],
                )

            # loads ordered to match consumption: DVE starts on block 0's
            # first quarter, PE on block 1's first half + its diag slice;
            # then blocks in the order the compute stream needs them.
            load_x(0, 0, Q + PAD)
            load_x(1, 0, CH + PAD)
            nc.sync.dma_start(out=wdt[:, 0 : K * 128], in_=wd[:, 0 : K * 128])
            load_x(0, Q + PAD, L + PAD)
            load_x(1, CH + PAD, L + PAD)
            nc.sync.dma_start(out=wdt[:, K * 128 :], in_=wd[:, K * 128 :])
            for blk in [3, 2, 5, 4, 6, 7]:
                load_x(blk, 0, L + PAD)

            def d_unit(blk, t0, tl, store_eng=None):
                # products shift-rebased (m_j[:, t] = w_j * x[:, t + j]) so
                # the fp16 add tree stays aligned; plain non-aliased adds.
                x = xtile[blk]
                r0 = blk * 128
                wj = lambda j: w[:, blk * K + j : blk * K + j + 1]
                m = pool.tile([128, 4, CH], MID_DT, tag="m", bufs=2)
                for j in range(K):
                    nc.vector.tensor_scalar_mul(
                        m[:, j, 0:tl], x[:, t0 + j : t0 + j + tl], wj(j)
                    )
                a = pool.tile([128, 3, CH], MID_DT, tag="a", bufs=4)
                nc.vector.tensor_add(a[:, 0, 0:tl], m[:, 0, 0:tl], m[:, 1, 0:tl])
                nc.vector.tensor_add(a[:, 1, 0:tl], m[:, 2, 0:tl], m[:, 3, 0:tl])
                nc.vector.tensor_add(a[:, 2, 0:tl], a[:, 0, 0:tl], a[:, 1, 0:tl])
                o = pool.tile([128, CH], MID_DT, tag="o", bufs=9)
                # silu in 1024-col pieces: uniform ACT granularity caps
                # head-of-line blocking of the PE path's PSUM drain
                for h in range(tl // 1024):
                    nc.scalar.activation(
                        o[:, h * 1024 : (h + 1) * 1024],
                        a[:, 2, h * 1024 : (h + 1) * 1024],
                        mybir.ActivationFunctionType.Silu,
                    )
                eng = store_eng or nc.gpsimd
                eng.dma_start(out=ot[r0 : r0 + 128, t0 : t0 + tl], in_=o[:, 0:tl])

            def p_unit(blk, t0, tl, store_eng=None):
                x = xtile[blk]
                r0 = blk * 128
                pi = P_BLKS.index(blk)
                # 1024-col PSUM tiles (2 banks) x 4 bufs: PE runs up to 4
                # tiles ahead of ACT's PSUM drain.
                o = pool.tile([128, CH], MID_DT, tag="o", bufs=9)
                for h in range(tl // 1024):
                    ps = psum_pool.tile([128, 1024], f32, tag="ps", bufs=4)
                    for c in range(2):
                        for j in range(K):
                            lw = wdt[:, (pi * K + j) * 128 : (pi * K + j + 1) * 128]
                            b0 = t0 + h * 1024 + c * 512
                            nc.tensor.matmul(
                                ps[:, c * 512 : (c + 1) * 512],
                                lw,
                                x[:, b0 + j : b0 + j + 512],
                                start=(j == 0),
                                stop=(j == K - 1),
                            )
                    nc.scalar.activation(
                        o[:, h * 1024 : (h + 1) * 1024],
                        ps[:],
                        mybir.ActivationFunctionType.Silu,
                    )
                eng = store_eng or nc.gpsimd
                eng.dma_start(out=ot[r0 : r0 + 128, t0 : t0 + tl], in_=o[:, 0:tl])

            # chunk stream ordered by estimated completion time (DVE ~3 us
            # per 1024 cols, PE ~1.85 us) so ACT's in-order SiLU queue
            # matches production order and neither engine head-of-line
            # blocks the other's drain. Block 0 starts with 1024-col units
            # so DVE ramps sooner; the last three stores issue on ACT HWDGE
            # (program-order adjacent to their SiLU) to shorten the drain.
            act = nc.scalar
            stream = [
                (d_unit, 0, 0, Q, None), (p_unit, 1, 0, CH, None),
                (d_unit, 0, Q, Q, None), (p_unit, 1, CH, CH, None),
                (d_unit, 0, CH, CH, None), (p_unit, 3, 0, CH, None),
                (p_unit, 3, CH, CH, None), (d_unit, 2, 0, CH, None),
                (p_unit, 5, 0, CH, None), (p_unit, 5, CH, CH, None),
                (d_unit, 2, CH, CH, None), (p_unit, 6, 0, CH, None),
                (d_unit, 4, 0, CH, None), (p_unit, 6, CH, CH, None),
                (p_unit, 7, 0, CH, act), (d_unit, 4, CH, CH, act),
                (p_unit, 7, CH, CH, act),
            ]
            for fn, blk, t0, tl, se in stream:
                fn(blk, t0, tl, se)
    nc.compile()
    return nc


def _shard_inputs(x, w):
    in_maps = []
    for core in range(N_CORES):
        b, half = divmod(core, 2)
        d0 = half * DH
        xt = np.zeros((DH, ROWW), dtype=np.float16)
        xt[:, PAD : PAD + L] = x[b, :, d0 : d0 + DH].T.astype(np.float16)
        # w rows for this shard, rearranged so partition p holds the K
        # weights of channel blk*128 + p at free cols [blk*K, blk*K + K)
        w_sh = w[d0 : d0 + DH].reshape(NBLK, 128, K)
        wt = (
            w_sh.transpose(1, 0, 2).reshape(128, NBLK * K).astype(np.float32)
        )
        # diag blocks for the PE path (P_BLKS only)
        wdv = np.zeros((128, len(P_BLKS), K, 128), dtype=np.float16)
        idx = np.arange(128)
        wdv[idx, :, :, idx] = w_sh[P_BLKS].transpose(1, 0, 2).astype(np.float16)
        in_maps.append(
            {
                "xt": np.ascontiguousarray(xt),
                "wt": np.ascontiguousarray(wt),
                "wd": np.ascontiguousarray(
                    wdv.reshape(128, len(P_BLKS) * K * 128)
                ),
            }
        )
    return in_maps


def kernel(x, w):
    x = np.asarray(x, dtype=np.float32)
    w = np.asarray(w, dtype=np.float32)
    assert x.shape == (B, L, D) and w.shape == (D, K)

    if "nc" not in _cache:
        _cache["nc"] = _build_bass()
    nc = _cache["nc"]

    in_maps = _shard_inputs(x, w)
    res = None
    for attempt in range(3):
        try:
            res = run_bass_kernel_spmd(nc, in_maps, core_ids=list(range(N_CORES)))
            break
        except Exception:
            if attempt == 2:
                raise
    _cache["last_results"] = res

    out = np.empty((B, L, D), dtype=np.float32)
    for core in range(N_CORES):
        b, half = divmod(core, 2)
        d0 = half * DH
        out[b, :, d0 : d0 + DH] = res.results[core]["ot"].T.astype(np.float32)
    return out
